# revision 6
# baseline (speedup 1.0000x reference)
"""Trainium2 Bass kernel for nn_Model_1580547969651.

Math (from the reference):
    s    = x @ sum(y, axis=0)          # (B,) row-sums of x @ y^T
    h    = hardswish(s)                # s * clip(s+3, 0, 6) / 6
    out  = clip(h + noise, -0.5, 0.5)  # (B, 1)

Strategy (v2): batch-shard x (core c owns rows [1024c, 1024c+1024)) and
column-shard y (core c owns features [512c, 512c+512)). Each core streams
its 16MB y slice first (whole 2MB super-tiles per DMA descriptor set,
16KB contiguous per partition), folding rows into a PSUM accumulator with
ones-matmuls on the idle TensorEngine. The local 512-feature ysum is then
AllGathered (2KB per core -> 16KB, the cheapest collective at this scale)
while the 16MB x slice streams behind it on the same queues. The gathered
ysum is broadcast to all 128 partitions via a rank-1 ones-matmul, and the
VectorEngine computes per-row dots with fused scalar_tensor_tensor ops as
x tiles land. Because every core only ever computes its own 1024 output
rows, there is NO end-of-kernel collective: the tail after the last x
byte is one quarter-tile dot, a 32x32 transpose, 5 tiny elementwise ops
and a 4KB store. A dummy 32B AllGather issued up front absorbs the ncfw
wake-up so the real AllGather starts promptly mid-stream.
"""

import numpy as np

from concourse import bass, bacc, mybir, tile
from concourse.bass_utils import run_bass_kernel_spmd

B = 8192
F = 4096
NCORES = 8
BL = B // NCORES        # 1024 output rows per core (x batch shard)
FL = F // NCORES        # 512 features per core (y column shard)
NYT = 8                 # y super-tiles: (128, 8, 512) = 2MB each
NSUB = 8                # y subtiles per super-tile
NXT = 8                 # x tiles: (128, 4096) = 2MB each
FP32 = mybir.dt.float32

_CACHE: dict = {}


def _build():
    nc = bacc.Bacc(
        "TRN2",
        target_bir_lowering=False,
        debug=False,
        num_devices=NCORES,
    )

    x_d = nc.dram_tensor("x", [BL, F], FP32, kind="ExternalInput")
    y_d = nc.dram_tensor("y", [B, FL], FP32, kind="ExternalInput")
    nz_d = nc.dram_tensor("noise", [BL, 1], FP32, kind="ExternalInput")
    out_d = nc.dram_tensor("out", [BL, 1], FP32, kind="ExternalOutput")

    # y: (s p c) packing -> partition p's slice of super-tile s is 8
    # consecutive DRAM rows = one contiguous 16KB chunk per descriptor.
    y_r = y_d[:, :].rearrange("(s p c) f -> s p c f", p=128, c=NSUB)
    # x: tile t, partition p = local row 128t+p -> 16KB contiguous.
    x_r = x_d[:, :].rearrange("(t p) f -> t p f", p=128)
    # noise/out in (t, p) layout: partition t holds 128 consecutive rows
    # = 512B contiguous per descriptor.
    nz_r = nz_d[:, 0].rearrange("(t p) -> t p", p=128)     # (8, 128)
    out_r = out_d[:, 0].rearrange("(t p) -> t p", p=128)   # (8, 128)

    with tile.TileContext(nc) as tc:
        with (
            tc.tile_pool(name="ypool", bufs=3) as ypool,
            tc.tile_pool(name="xpool", bufs=6) as xpool,
            tc.tile_pool(name="small", bufs=1) as small,
            tc.tile_pool(name="scratch", bufs=1) as scratch,
            tc.tile_pool(name="psum_a", bufs=1, space="PSUM") as psum_a,
            tc.tile_pool(name="psum_b", bufs=1, space="PSUM") as psum_b,
            tc.tile_pool(name="dram", bufs=1, space="DRAM") as dram,
        ):
            ones128 = small.tile([128, 128], FP32)
            nc.gpsimd.memset(ones128[:], 1.0)

            # tiny dummy collective, issued up front: pays the ncfw wake +
            # entry rendezvous while the y stream runs, so the real
            # AllGather mid-kernel starts without the first-op delay
            warm = small.tile([1, 8], FP32)
            nc.gpsimd.memset(warm[:], 0.0)
            warm_in = dram.tile([8], FP32)
            warm_out = dram.tile([8 * NCORES], FP32)
            nc.gpsimd.dma_start(warm_in[:], warm[:])
            nc.gpsimd.collective_compute(
                "AllGather",
                mybir.AluOpType.bypass,
                replica_groups=[list(range(NCORES))],
                ins=[warm_in.opt()],
                outs=[warm_out.opt()],
            )

            # noise is only needed at the very end; load it now on the
            # (otherwise idle) SWDGE queue
            noise_t = small.tile([NXT, 128], FP32)
            nc.gpsimd.dma_start(noise_t[:], nz_r)

            # ---- phase Y: stream the 16MB y column-slice, reduce rows on
            # the TensorEngine: bc_loc[q, f] = sum_p ones[p, q]*ytile[p,c,f]
            # accumulated over all 64 subtiles ----
            bc_loc = psum_a.tile([128, FL], FP32, tag="bcl")
            for s in range(NYT):
                ytile = ypool.tile([128, NSUB, FL], FP32, tag="y")
                q = nc.sync if s % 2 == 0 else nc.scalar
                q.dma_start(ytile[:], y_r[s])
                for c in range(NSUB):
                    nc.tensor.matmul(
                        bc_loc[:], ones128[:], ytile[:, c, :],
                        start=(s == 0 and c == 0),
                        stop=(s == NYT - 1 and c == NSUB - 1),
                    )

            # local ysum slice (row 0; all 128 rows are identical)
            ysum_row = small.tile([1, FL], FP32)
            nc.vector.tensor_copy(ysum_row[:], bc_loc[0:1, :])

            # ---- AllGather the 2KB ysum slice -> full 16KB ysum ----
            cc_in = dram.tile([FL], FP32)
            cc_out = dram.tile([F], FP32)
            nc.gpsimd.dma_start(cc_in[:].rearrange("(a f) -> a f", a=1),
                                ysum_row[:])
            nc.gpsimd.collective_compute(
                "AllGather",
                mybir.AluOpType.bypass,
                replica_groups=[list(range(NCORES))],
                ins=[cc_in.opt()],
                outs=[cc_out.opt()],
            )
            ys_full = small.tile([1, F], FP32)
            nc.gpsimd.dma_start(ys_full[:],
                                cc_out[:].rearrange("(a f) -> a f", a=1))

            # broadcast ysum to all 128 partitions via rank-1 ones-matmul,
            # in two (128, 2048) halves (4 PSUM banks each, reused)
            bc_sb = small.tile([128, F], FP32)
            for h in range(2):
                bc_ps = psum_b.tile([128, F // 2], FP32, tag="bcb")
                for j in range(4):
                    lo = 512 * j
                    nc.tensor.matmul(
                        bc_ps[:, lo:lo + 512],
                        ones128[0:1, :],
                        ys_full[0:1, 2048 * h + lo:2048 * h + lo + 512],
                        start=True, stop=True,
                    )
                nc.vector.tensor_copy(bc_sb[:, 2048 * h:2048 * (h + 1)],
                                      bc_ps[:])

            # ---- phase X: stream the 16MB x row-slice; fused dot per tile
            # s_part[p, t] = sum_f x[128t+p, f] * ysum[f] ----
            sp = small.tile([128, 32], FP32)
            # one shared scratch for the mandatory (unread) STT out; DVE
            # ops serialize in-order so reuse costs nothing
            prod = scratch.tile([128, F], FP32, tag="sc")
            for t in range(NXT):
                xtile = xpool.tile([128, F], FP32, tag="x")
                q = nc.sync if t % 2 == 0 else nc.scalar
                if t < NXT - 1:
                    q.dma_start(xtile[:], x_r[t])
                    nc.vector.scalar_tensor_tensor(
                        out=prod[:],
                        in0=xtile[:],
                        scalar=1.0,
                        in1=bc_sb[:],
                        op0=mybir.AluOpType.mult,
                        op1=mybir.AluOpType.mult,
                        accum_out=sp[:, t:t + 1],
                    )
                else:
                    # last tile in 4 quarter-chunks so only ~1.3us of dot
                    # trails the final DMA arrival
                    for k in range(4):
                        qq = nc.sync if k % 2 == 0 else nc.scalar
                        qq.dma_start(xtile[:, 1024 * k:1024 * (k + 1)],
                                     x_r[t][:, 1024 * k:1024 * (k + 1)])
                    for k in range(4):
                        nc.vector.scalar_tensor_tensor(
                            out=prod[:, 0:1024],
                            in0=xtile[:, 1024 * k:1024 * (k + 1)],
                            scalar=1.0,
                            in1=bc_sb[:, 1024 * k:1024 * (k + 1)],
                            op0=mybir.AluOpType.mult,
                            op1=mybir.AluOpType.mult,
                            accum_out=sp[:, 8 + k:9 + k],
                        )
            # fold the 4 quarter-dots of the last tile into column 7
            nc.vector.tensor_tensor(
                out=sp[:, 12:14], in0=sp[:, 8:10], in1=sp[:, 10:12],
                op=mybir.AluOpType.add,
            )
            nc.vector.tensor_tensor(
                out=sp[:, 7:8], in0=sp[:, 12:13], in1=sp[:, 13:14],
                op=mybir.AluOpType.add,
            )

            # ---- transpose s to (t, p) layout (32x32 DVE blocks; only
            # rows 0..7 of the result are meaningful) ----
            s_t = small.tile([32, 128], FP32)
            for i in range(4):
                nc.vector.transpose(
                    s_t[0:32, 32 * i:32 * (i + 1)],
                    sp[32 * i:32 * (i + 1), 0:32],
                )

            # ---- tail: hardswish, + noise, hardtanh on (8, 128) ----
            t_ = small.tile([NXT, 128], FP32)
            nc.vector.tensor_scalar(
                out=t_[:], in0=s_t[0:NXT, :], scalar1=3.0, scalar2=0.0,
                op0=mybir.AluOpType.add, op1=mybir.AluOpType.max,
            )
            nc.vector.tensor_scalar(
                out=t_[:], in0=t_[:], scalar1=6.0, scalar2=1.0 / 6.0,
                op0=mybir.AluOpType.min, op1=mybir.AluOpType.mult,
            )
            r = small.tile([NXT, 128], FP32)
            nc.vector.tensor_tensor(
                out=r[:], in0=s_t[0:NXT, :], in1=t_[:],
                op=mybir.AluOpType.mult,
            )
            nc.vector.tensor_tensor(
                out=r[:], in0=r[:], in1=noise_t[:], op=mybir.AluOpType.add,
            )
            nc.vector.tensor_scalar(
                out=r[:], in0=r[:], scalar1=-0.5, scalar2=0.5,
                op0=mybir.AluOpType.max, op1=mybir.AluOpType.min,
            )
            nc.sync.dma_start(out_r, r[:])

    nc.compile()
    return nc


def _get_nc():
    if "nc" not in _CACHE:
        _CACHE["nc"] = _build()
    return _CACHE["nc"]


def kernel(x: np.ndarray, y: np.ndarray, noise: np.ndarray, **_run_kwargs) -> np.ndarray:
    x = np.ascontiguousarray(x, dtype=np.float32)
    y = np.ascontiguousarray(y, dtype=np.float32)
    noise = np.ascontiguousarray(noise, dtype=np.float32)

    nc = _get_nc()
    in_maps = [
        {
            "x": np.ascontiguousarray(x[i * BL:(i + 1) * BL, :]),
            "y": np.ascontiguousarray(y[:, i * FL:(i + 1) * FL]),
            "noise": np.ascontiguousarray(noise[i * BL:(i + 1) * BL, :]),
        }
        for i in range(NCORES)
    ]
    res = run_bass_kernel_spmd(nc, in_maps, list(range(NCORES)), **_run_kwargs)
    out = np.concatenate(
        [res.results[i]["out"] for i in range(NCORES)], axis=0,
    )
    if _run_kwargs:
        _CACHE["last_results"] = res
    return out


# revision 14
# speedup vs baseline: 1.3779x; 1.3779x over previous
"""Trainium2 Bass kernel for nn_Model_1580547969651.

Math (from the reference):
    s    = x @ sum(y, axis=0)          # (B,) row-sums of x @ y^T
    h    = hardswish(s)                # s * clip(s+3, 0, 6) / 6
    out  = clip(h + noise, -0.5, 0.5)  # (B, 1)

Strategy (v2): batch-shard x (core c owns rows [1024c, 1024c+1024)) and
column-shard y (core c owns features [512c, 512c+512)). Each core streams
its 16MB y slice first (whole 2MB super-tiles per DMA descriptor set,
16KB contiguous per partition), folding rows into a PSUM accumulator with
ones-matmuls on the idle TensorEngine. The local 512-feature ysum is then
AllGathered (2KB per core -> 16KB, the cheapest collective at this scale)
while the 16MB x slice streams behind it on the same queues. The gathered
ysum is broadcast to all 128 partitions via a rank-1 ones-matmul, and the
VectorEngine computes per-row dots with fused scalar_tensor_tensor ops as
x tiles land. Because every core only ever computes its own 1024 output
rows, there is NO end-of-kernel collective: the tail after the last x
byte is one quarter-tile dot, a 32x32 transpose, 5 tiny elementwise ops
and a 4KB store. A dummy 32B AllGather issued up front absorbs the ncfw
wake-up so the real AllGather starts promptly mid-stream.
"""

import numpy as np

from concourse import bass, bacc, mybir, tile
from concourse.bass_utils import run_bass_kernel_spmd

B = 8192
F = 4096
NCORES = 8
BL = B // NCORES        # 1024 output rows per core (x batch shard)
FL = F // NCORES        # 512 features per core (y column shard)
NYT = 8                 # y super-tiles: (128, 8, 512) = 2MB each
NSUB = 8                # y subtiles per super-tile
NXT = 8                 # x tiles: (128, 4096) = 2MB each
FP32 = mybir.dt.float32

_CACHE: dict = {}


def _build():
    nc = bacc.Bacc(
        "TRN2",
        target_bir_lowering=False,
        debug=False,
        num_devices=NCORES,
    )

    x_d = nc.dram_tensor("x", [BL, F], FP32, kind="ExternalInput")
    y_d = nc.dram_tensor("y", [B, FL], FP32, kind="ExternalInput")
    nz_d = nc.dram_tensor("noise", [BL, 1], FP32, kind="ExternalInput")
    out_d = nc.dram_tensor("out", [BL, 1], FP32, kind="ExternalOutput")

    # y: (s p c) packing -> partition p's slice of super-tile s is 8
    # consecutive DRAM rows = one contiguous 16KB chunk per descriptor.
    y_r = y_d[:, :].rearrange("(s p c) f -> s p c f", p=128, c=NSUB)
    # x: tile t, partition p = local row 128t+p -> 16KB contiguous.
    x_r = x_d[:, :].rearrange("(t p) f -> t p f", p=128)
    # noise/out in (t, p) layout: partition t holds 128 consecutive rows
    # = 512B contiguous per descriptor.
    nz_r = nz_d[:, 0].rearrange("(t p) -> t p", p=128)     # (8, 128)
    out_r = out_d[:, 0].rearrange("(t p) -> t p", p=128)   # (8, 128)

    with tile.TileContext(nc) as tc:
        with (
            tc.tile_pool(name="ypool", bufs=3) as ypool,
            tc.tile_pool(name="xpool", bufs=5) as xpool,
            tc.tile_pool(name="small", bufs=1) as small,
            tc.tile_pool(name="scratch", bufs=1) as scratch,
            tc.tile_pool(name="psum_a", bufs=1, space="PSUM") as psum_a,
            tc.tile_pool(name="dram", bufs=1, space="DRAM") as dram,
        ):
            ones128 = small.tile([128, 128], FP32)
            nc.gpsimd.memset(ones128[:], 1.0)

            # tiny dummy collective, issued up front: pays the ncfw wake +
            # entry rendezvous while the y stream runs, so the real
            # AllGather mid-kernel starts without the first-op delay.
            # warm_in goes out on the sync HWDGE queue BEFORE the y stream
            # so the doorbell rings at ~9us instead of ~20us.
            warm = small.tile([1, 8], FP32)
            nc.gpsimd.memset(warm[:], 0.0)
            warm_in = dram.tile([8], FP32)
            warm_out = dram.tile([8 * NCORES], FP32)
            nc.sync.dma_start(warm_in[:].rearrange("(a f) -> a f", a=1),
                              warm[:])
            nc.gpsimd.collective_compute(
                "AllGather",
                mybir.AluOpType.bypass,
                replica_groups=[list(range(NCORES))],
                ins=[warm_in.opt()],
                outs=[warm_out.opt()],
            )

            # noise is only needed at the very end; load it now on the
            # (otherwise idle) SWDGE queue
            noise_t = small.tile([NXT, 128], FP32)
            nc.gpsimd.dma_start(noise_t[:], nz_r)

            # ---- phase Y: stream the 16MB y column-slice. fp32 PE matmuls
            # run in a slow LOW/HIGH two-pass mode (~1.2us each), so fold
            # 8 subtiles -> 2 on the DVE first and only feed 2 matmuls per
            # super-tile into the PSUM accumulator ----
            bc_loc = psum_a.tile([128, FL], FP32, tag="bcl")
            for s in range(NYT):
                ytile = ypool.tile([128, NSUB, FL], FP32, tag="y")
                q = nc.sync if s % 2 == 0 else nc.scalar
                q.dma_start(ytile[:], y_r[s])
                nc.vector.tensor_add(ytile[:, 0:4, :], ytile[:, 0:4, :],
                                     ytile[:, 4:8, :])
                nc.vector.tensor_add(ytile[:, 0:2, :], ytile[:, 0:2, :],
                                     ytile[:, 2:4, :])
                for c in range(2):
                    nc.tensor.matmul(
                        bc_loc[:], ones128[:], ytile[:, c, :],
                        start=(s == 0 and c == 0),
                        stop=(s == NYT - 1 and c == 1),
                    )

            # local ysum slice (row 0; all 128 rows are identical)
            ysum_row = small.tile([1, FL], FP32)
            nc.vector.tensor_copy(ysum_row[:], bc_loc[0:1, :])

            # ---- AllGather the 2KB ysum slice -> full 16KB ysum ----
            cc_in = dram.tile([FL], FP32)
            cc_out = dram.tile([F], FP32)
            nc.gpsimd.dma_start(cc_in[:].rearrange("(a f) -> a f", a=1),
                                ysum_row[:])
            nc.gpsimd.collective_compute(
                "AllGather",
                mybir.AluOpType.bypass,
                replica_groups=[list(range(NCORES))],
                ins=[cc_in.opt()],
                outs=[cc_out.opt()],
            )
            ys_full = small.tile([1, F], FP32)
            nc.gpsimd.dma_start(ys_full[:],
                                cc_out[:].rearrange("(a f) -> a f", a=1))

            # broadcast ysum to all 128 partitions (gpsimd DMA broadcast)
            bc_sb = small.tile([128, F], FP32)
            nc.gpsimd.partition_broadcast(bc_sb[:], ys_full[:])

            # ---- phase X: stream the 16MB x row-slice; fused dot per tile
            # s_part[p, t] = sum_f x[128t+p, f] * ysum[f] ----
            sp = small.tile([128, 32], FP32)
            # shared scratch for the mandatory (unread) STT out; DVE ops
            # serialize in-order so reuse costs nothing
            prod = scratch.tile([128, F], FP32, tag="sc")

            def dot(eng, pr, x_ap, bc_ap, col):
                eng.scalar_tensor_tensor(
                    out=pr,
                    in0=x_ap,
                    scalar=1.0,
                    in1=bc_ap,
                    op0=mybir.AluOpType.mult,
                    op1=mybir.AluOpType.mult,
                    accum_out=sp[:, col:col + 1],
                )

            for t in range(NXT):
                xtile = xpool.tile([128, F], FP32, tag="x")
                q = nc.sync if t % 2 == 0 else nc.scalar
                if t < NXT - 1:
                    q.dma_start(xtile[:], x_r[t])
                    dot(nc.vector, prod[:], xtile[:], bc_sb[:], t)
                else:
                    # last tile in 4 quarter-chunks so only ~1.3us of dot
                    # trails the final DMA arrival (STT is DVE-only: the
                    # Pool engine rejects InstTensorScalarPtr)
                    for k in range(4):
                        qq = nc.sync if k % 2 == 0 else nc.scalar
                        qq.dma_start(xtile[:, 1024 * k:1024 * (k + 1)],
                                     x_r[t][:, 1024 * k:1024 * (k + 1)])
                    for k in range(4):
                        dot(nc.vector, prod[:, 0:1024],
                            xtile[:, 1024 * k:1024 * (k + 1)],
                            bc_sb[:, 1024 * k:1024 * (k + 1)],
                            8 + k)
            # fold the 4 quarter-dots of the last tile into column 7
            nc.vector.tensor_tensor(
                out=sp[:, 12:14], in0=sp[:, 8:10], in1=sp[:, 10:12],
                op=mybir.AluOpType.add,
            )
            nc.vector.tensor_tensor(
                out=sp[:, 7:8], in0=sp[:, 12:13], in1=sp[:, 13:14],
                op=mybir.AluOpType.add,
            )

            # ---- transpose s to (t, p) layout (32x32 DVE blocks; only
            # rows 0..7 of the result are meaningful) ----
            s_t = small.tile([32, 128], FP32)
            for i in range(4):
                nc.vector.transpose(
                    s_t[0:32, 32 * i:32 * (i + 1)],
                    sp[32 * i:32 * (i + 1), 0:32],
                )

            # ---- tail: hardswish, + noise, hardtanh on (8, 128) ----
            t_ = small.tile([NXT, 128], FP32)
            nc.vector.tensor_scalar(
                out=t_[:], in0=s_t[0:NXT, :], scalar1=3.0, scalar2=0.0,
                op0=mybir.AluOpType.add, op1=mybir.AluOpType.max,
            )
            nc.vector.tensor_scalar(
                out=t_[:], in0=t_[:], scalar1=6.0, scalar2=1.0 / 6.0,
                op0=mybir.AluOpType.min, op1=mybir.AluOpType.mult,
            )
            r = small.tile([NXT, 128], FP32)
            nc.vector.tensor_tensor(
                out=r[:], in0=s_t[0:NXT, :], in1=t_[:],
                op=mybir.AluOpType.mult,
            )
            nc.vector.tensor_tensor(
                out=r[:], in0=r[:], in1=noise_t[:], op=mybir.AluOpType.add,
            )
            nc.vector.tensor_scalar(
                out=r[:], in0=r[:], scalar1=-0.5, scalar2=0.5,
                op0=mybir.AluOpType.max, op1=mybir.AluOpType.min,
            )
            nc.sync.dma_start(out_r, r[:])

    nc.compile()
    return nc


def _get_nc():
    if "nc" not in _CACHE:
        _CACHE["nc"] = _build()
    return _CACHE["nc"]


def kernel(x: np.ndarray, y: np.ndarray, noise: np.ndarray, **_run_kwargs) -> np.ndarray:
    x = np.ascontiguousarray(x, dtype=np.float32)
    y = np.ascontiguousarray(y, dtype=np.float32)
    noise = np.ascontiguousarray(noise, dtype=np.float32)

    nc = _get_nc()
    in_maps = [
        {
            "x": np.ascontiguousarray(x[i * BL:(i + 1) * BL, :]),
            "y": np.ascontiguousarray(y[:, i * FL:(i + 1) * FL]),
            "noise": np.ascontiguousarray(noise[i * BL:(i + 1) * BL, :]),
        }
        for i in range(NCORES)
    ]
    res = run_bass_kernel_spmd(nc, in_maps, list(range(NCORES)), **_run_kwargs)
    out = np.concatenate(
        [res.results[i]["out"] for i in range(NCORES)], axis=0,
    )
    if _run_kwargs:
        _CACHE["last_results"] = res
    return out
